# revision 66
# baseline (speedup 1.0000x reference)
"""EntropyProfileLoss Trainium2 kernel — TensorE windowed-sum design.

Math: for a window t of length k, sum(softmax(t)*log_softmax(t))
      = S2/S1 - ln(S1),  S1 = sum(exp(t)), S2 = sum(t*exp(t)).

Layout: pure data parallel over batch B=64 -> 8 cores x 8 batches
(16 rows of L=2048 per core).  Each row is split into 16 blocks of
128; the host transposes to a [128, 257] SBUF image whose partition
axis is the position-within-block j and whose free axis is
c = 16*row + block (col 256 is a -100 pad so exp() gives 0).  In this
layout a window sum starting at (j, c) is a sum down the partition
axis spilling into column c+1, which TensorE computes as two banded
matmuls per window size:

  S_k[q, c] = sum_{p=q}^{q+k-1} E[p, c] + sum_{p=0}^{q+k-129} E[p, c+1]
            = (A_k^T E)[q, c]          + (B_k^T E_next)[q, c]

with A_k[p,q] = 1_{q <= p < q+k}, B_k[p,q] = 1_{p < q+k-128} packed as
12 bf16 [128,128] weights (host inputs).  E = exp(x) and XE = x*E are
bf16; the matmuls accumulate S1 (from E) and S2 (from XE) in fp32
PSUM — one 1024-col region [S1x|S2x|S1t|S2t] per k, cycled through
the 8 PSUM banks (4 k's in flight).

Downstream per k: U = ln(S1) (ACT, PSUM-src, fp32), R = exp(-U) bf16
(ACT), D = S2*R (DVE, PSUM-src — frees the PSUM buffer), dU = Ux-Ut
(GpSimd).  Pair tails (dD/dx bf16 2x + |dx| sums via
tensor_reduce(apply_absolute_value) into ACC) are deferred behind
later D-mults in the vector queue so the matmul stream never stalls
on PSUM reuse.  Invalid windows (block 15, q >= 129-k run past the
row end) are excluded by splitting the reduce: blocks 0-14 fully,
block 15 only over its valid partitions.  The host reduces ACC over
cores/partitions and applies the per-k 1/(B*C*(L-k+1)) mean scaling.

Startup: k=4,8 weights are generated on-chip (affine_select — any
weight DMA costs a fixed ~3.5us since DMA is one packet per
partition); x/t are split across both HWDGE queues (sync + scalar)
with the k>=16 weights behind them; dummy matmuls on a zero tile keep
the PE busy during the DMA wait so real matmuls run unthrottled
(HAM); a dummy activation anchors the single
natural_log_exp_and_others ACT table load (see _patch_act_tables)
ahead of the scalar-queue DMA issues.
"""

import sys

import numpy as np

if "/opt/trn_rl_repo" not in sys.path:
    sys.path.insert(0, "/opt/trn_rl_repo")

import ml_dtypes

import concourse.bacc as bacc
import concourse.bass as bass
import concourse.tile as tile
from concourse import mybir

KERNELS = (4, 8, 16, 32, 64, 128)
NK = len(KERNELS)
B, C, L = 64, 2, 2048
N_CORES = 8
ROWS = (B // N_CORES) * C          # 16 rows per core
NB = L // 128                      # 16 blocks per row
NBP = NB + 1                       # +1 pad block-col of +30 per row
COLS = ROWS * NB                   # 256 valid window-start columns
NCOL = ROWS * NBP                  # 272 (free dim: c = 17*row + block)
PAD = 30.0                         # e^30 ~ 1e13: invalid windows (block 15
                                   # runs past the row end) read the pad col
                                   # and their S1/S2 are pad-dominated ->
                                   # bit-identical for x and t -> dx = 0

F32 = mybir.dt.float32
BF16 = mybir.dt.bfloat16
AF = mybir.ActivationFunctionType
OP = mybir.AluOpType

_CACHE: dict = {}


def _patch_act_tables():
    """Keep Exp/Ln resolvable only via natural_log_exp_and_others so the
    table-load pass emits one ACT table set (one ~2.7us load)."""
    if _CACHE.get("act_patched"):
        return
    orig = bacc.get_activation_tables
    funcs = {AF.Exp, AF.Ln, AF.Abs}

    def patched(arch):
        tables = dict(orig(arch))
        return {
            name: (fs if name == "natural_log_exp_and_others" else fs - funcs)
            for name, fs in tables.items()
        }

    bacc.get_activation_tables = patched
    _CACHE["act_patched"] = True


def make_weights() -> np.ndarray:
    """[128, 2*NK, 128] bf16: A_k, B_k interleaved in k order."""
    p = np.arange(128)[:, None]
    q = np.arange(128)[None, :]
    w = np.zeros((128, 2 * NK, 128), dtype=np.float32)
    for ki, k in enumerate(KERNELS):
        w[:, 2 * ki, :] = ((q <= p) & (p < q + k)).astype(np.float32)
        w[:, 2 * ki + 1, :] = (p < q + k - 128).astype(np.float32)
    return w.astype(ml_dtypes.bfloat16)


def build(debug: bool = False, xe_on_gpsimd: bool = False,
          du_on_gpsimd: bool = True, warm_mms: int = 34):
    _patch_act_tables()
    nc = bacc.Bacc("TRN2", target_bir_lowering=False)

    x_d = nc.dram_tensor("x", [128, NCOL], F32, kind="ExternalInput")
    t_d = nc.dram_tensor("t", [128, NCOL], F32, kind="ExternalInput")
    w_d = nc.dram_tensor("w", [128, 2 * NK * 128], BF16, kind="ExternalInput")
    acc_d = nc.dram_tensor("acc", [128, 2 * NK], F32, kind="ExternalOutput")
    if debug:
        ex_d = nc.dram_tensor("dbg_ex", [128, 4 * NCOL], BF16,
                              kind="ExternalOutput")
        u_d = nc.dram_tensor("dbg_u", [128, NK * 2 * COLS], F32,
                             kind="ExternalOutput")
        dx_d = nc.dram_tensor("dbg_dx", [128, NK * COLS], BF16,
                              kind="ExternalOutput")
        ps_d = nc.dram_tensor("dbg_ps", [128, NK * 1024], F32,
                              kind="ExternalOutput")
        d_d = nc.dram_tensor("dbg_d", [128, NK * 2 * COLS], BF16,
                             kind="ExternalOutput")

    with tile.TileContext(nc) as tc:
        with (
            tc.tile_pool(name="big", bufs=1) as big,
            tc.tile_pool(name="ps", bufs=4, space="PSUM") as psp,
        ):
            X = big.tile([128, 2, NCOL], F32)          # [x | t]
            EX = big.tile([128, 2, 2, NCOL], BF16)     # [tensor][E | XE]
            W = big.tile([128, 2 * NK, 128], BF16)
            U = big.tile([128, NK, 2, COLS], F32)
            S2S = big.tile([128, 3, 2, COLS], BF16)
            R = big.tile([128, NK, 2, COLS], BF16)
            D = big.tile([128, NK, 2, COLS], BF16)
            dU = big.tile([128, NK, COLS], BF16)
            dD = big.tile([128, NK, COLS], BF16)
            dx = big.tile([128, NK, COLS], BF16)
            ACC = big.tile([128, 2, NK], F32)          # [main | block-15 tail]
            ps_first = psp.tile([128, 1024], F32, name="ps_k")

            WARM = big.tile([128, 128], BF16)
            DUMO = big.tile([128, 1], F32)
            # dummy activation: anchors the ACT table load ahead of the
            # scalar-queue DMA issues, so exp isn't gated on a late load
            nc.vector.memset(WARM[:, :], 0.0)
            nc.scalar.activation(out=DUMO[:, :], in_=WARM[:, 0:1], func=AF.Exp)
            nc.vector.memset(ACC[:, :, :], 0.0)
            PSC = big.tile([128, NK, 1024], F32, name="PSC") if debug else None

            # ---- k=4,8 weights generated on-chip (a weight DMA costs a
            # fixed ~3.5us: one packet per partition); k>=16 weights DMA'd
            # behind x/t split across the two HWDGE queues
            ONESW = big.tile([128, 128], BF16)
            TMPW = big.tile([128, 128], BF16)
            nc.gpsimd.memset(ONESW[:, :], 1.0)
            for wi, k in ((0, 4), (1, 8)):
                # A_k: 1 where 0 <= p-q <= k-1
                nc.gpsimd.affine_select(
                    out=TMPW[:, :], in_=ONESW[:, :], pattern=[[-1, 128]],
                    compare_op=OP.is_ge, fill=0.0, base=0, channel_multiplier=1,
                )
                # (k-1) - (p-q) >= 0  (i.e. p-q <= k-1)
                nc.gpsimd.affine_select(
                    out=W[:, 2 * wi, :], in_=TMPW[:, :], pattern=[[1, 128]],
                    compare_op=OP.is_ge, fill=0.0, base=k - 1,
                    channel_multiplier=-1,
                )
                # B_k: q + k - 129 - p >= 0  (i.e. p < q+k-128)
                nc.gpsimd.affine_select(
                    out=W[:, 2 * wi + 1, :], in_=ONESW[:, :],
                    pattern=[[1, 128]], compare_op=OP.is_ge, fill=0.0,
                    base=k - 129, channel_multiplier=-1,
                )
            wv = W[:, :, :].rearrange("p a b -> p (a b)")
            nc.sync.dma_start(out=X[0:64, 0, :], in_=x_d[0:64, :])
            nc.scalar.dma_start(out=X[64:128, 0, :], in_=x_d[64:128, :])
            nc.sync.dma_start(out=X[0:64, 1, :], in_=t_d[0:64, :])
            nc.scalar.dma_start(out=X[64:128, 1, :], in_=t_d[64:128, :])
            nc.sync.dma_start(
                out=wv[:, 4 * 128 : 8 * 128], in_=w_d[:, 4 * 128 : 8 * 128]
            )
            nc.scalar.dma_start(out=wv[:, 8 * 128 :], in_=w_d[:, 8 * 128 :])

            # HAM warmup: keep the PE busy during the DMA wait so the real
            # matmuls run at 2.4 GHz (unthrottled); scratch area inside the
            # k=4 tile — its start=True matmul clears the bank anyway
            for _ in range(warm_mms):
                nc.tensor.matmul(
                    ps_first[:, 896:1024], WARM[:, :], WARM[:, :],
                    start=True, stop=True,
                )

            # ---- E = exp(x) bf16, XE = x * E bf16 (pad col -> 0) ----
            xe_eng = nc.gpsimd if xe_on_gpsimd else nc.vector
            for a in range(2):
                # split by the DMA partition halves so each half's exp/XE
                # starts as soon as its own queue's transfer lands
                for lo, hi in ((0, 64), (64, 128)):
                    nc.scalar.activation(
                        out=EX[lo:hi, a, 0, :], in_=X[lo:hi, a, :],
                        func=AF.Exp,
                    )
                    xe_eng.tensor_tensor(
                        out=EX[lo:hi, a, 1, :], in0=X[lo:hi, a, :],
                        in1=EX[lo:hi, a, 0, :], op=OP.mult,
                    )

            def emit_tail(ki):
                # dD/dx and the |dx| sums for the (ki-1, ki) pair, skipping
                # invalid windows (block 15 windows with q >= 129-k run past
                # the row end): main reduce over blocks 0-14, tail reduce
                # over block 15's valid rows
                nc.vector.tensor_tensor(
                    out=dD[:, ki - 1 : ki + 1],
                    in0=D[:, ki - 1 : ki + 1, 0],
                    in1=D[:, ki - 1 : ki + 1, 1],
                    op=OP.subtract,
                )
                nc.vector.tensor_tensor(
                    out=dx[:, ki - 1 : ki + 1],
                    in0=dD[:, ki - 1 : ki + 1],
                    in1=dU[:, ki - 1 : ki + 1],
                    op=OP.subtract,
                )
                # invalid windows read the +30 pad col and cancel exactly,
                # so one fused |dx| reduce covers everything
                nc.vector.tensor_reduce(
                    out=ACC[:, 0, ki - 1 : ki + 1],
                    in_=dx[:, ki - 1 : ki + 1, :],
                    axis=mybir.AxisListType.X,
                    op=OP.add,
                    apply_absolute_value=True,
                )

            def emit_rd(kj, s2):
                nc.scalar.activation(
                    out=R[:, kj], in_=U[:, kj], func=AF.Exp, scale=-1.0,
                )
                nc.vector.tensor_tensor(
                    out=D[:, kj], in0=s2, in1=R[:, kj], op=OP.mult,
                )

            # ---- per window size: banded matmuls + entropy pipeline ----
            rd_queue = []
            late_rd = []
            for ki, k in enumerate(KERNELS):
                ps_k = ps_first if ki == 0 else psp.tile(
                    [128, 1024], F32, name="ps_k")
                reg = ps_k[:, :].rearrange(
                    "p (a s c) -> p a s c", a=2, s=2
                )  # [tensor][S1|S2][256]
                # one matmul per (weight, tensor) covering [E | XE] — FD 512
                # fills the bank exactly, so each bank sees one start/stop
                for wi, off, start in ((2 * ki, 0, True), (2 * ki + 1, 1, False)):
                    for a in range(2):
                        exv = EX[:, a, :, :].rearrange(
                            "p s (r b) -> p s r b", b=NBP
                        )
                        nc.tensor.matmul(
                            reg[:, a, :, :],
                            W[:, wi, :],
                            exv[:, :, :, off : off + NB],
                            start=start,
                            stop=not start,
                        )
                if debug:
                    nc.vector.tensor_copy(out=PSC[:, ki], in_=ps_k[:, :])
                nc.scalar.activation(
                    out=U[:, ki], in_=reg[:, :, 0, :], func=AF.Ln
                )
                du_eng = nc.gpsimd if du_on_gpsimd else nc.vector
                du_eng.tensor_tensor(
                    out=dU[:, ki], in0=U[:, ki, 0], in1=U[:, ki, 1],
                    op=OP.subtract,
                )
                # For the first three k's (which hold all PSUM buffers while
                # later matmuls queue), drain S2 to SBUF right after the
                # matmuls — the vector engine is idle here and the buffer
                # then frees after LN, unblocking the k>=32 matmul stream.
                # Their R/D (which don't touch PSUM) are deferred one
                # iteration so the buffer release isn't held back by them
                # (pool release is emission-order based).  Later k's
                # multiply straight from PSUM.
                if ki < 3:
                    nc.vector.tensor_copy(out=S2S[:, ki], in_=reg[:, :, 1, :])
                src = S2S[:, ki] if ki < 3 else reg[:, :, 1, :]
                for kj, s2 in rd_queue:
                    emit_rd(kj, s2)
                rd_queue = []
                if ki in (2, 3):
                    # k16/k32 R+D only feed tail 1 — run them after the
                    # last pair's LN->R->D so the end chain isn't queued
                    # behind them on ACT (k32's PSUM buffer is never reused)
                    late_rd.append((ki, src))
                elif ki < 2:
                    rd_queue.append((ki, src))
                else:
                    emit_rd(ki, src)
                # defer pair tails so every D-mult (which frees a PSUM
                # buffer and unblocks matmuls) stays ahead of tail work in
                # the vector queue: tail 0 lands after D-k32, tails 1+2
                # after the last D
                if ki == 3:
                    emit_tail(1)
            emit_tail(5)
            for kj, s2 in late_rd:
                emit_rd(kj, s2)
            emit_tail(3)
            nc.sync.dma_start(
                out=acc_d[:, :],
                in_=ACC[:, :, :].rearrange("p a k -> p (a k)"),
            )
            if debug:
                nc.sync.dma_start(
                    out=ex_d[:, :],
                    in_=EX[:, :, :, :].rearrange("p a s c -> p (a s c)"),
                )
                nc.sync.dma_start(
                    out=u_d[:, :],
                    in_=U[:, :, :, :].rearrange("p k a c -> p (k a c)"),
                )
                nc.sync.dma_start(
                    out=dx_d[:, :],
                    in_=dx[:, :, :].rearrange("p k c -> p (k c)"),
                )
                nc.sync.dma_start(
                    out=ps_d[:, :],
                    in_=PSC[:, :, :].rearrange("p k c -> p (k c)"),
                )
                nc.sync.dma_start(
                    out=d_d[:, :],
                    in_=D[:, :, :, :].rearrange("p k a c -> p (k a c)"),
                )

    nc.compile()
    return nc


def make_runner(nc):
    """Once-jitted 8-core runner (run_bass_via_pjrt re-traces per call)."""
    import jax
    from jax.sharding import Mesh, PartitionSpec
    from jax.experimental.shard_map import shard_map
    from concourse import bass2jax
    from concourse import mybir as mb

    bass2jax.install_neuronx_cc_hook()

    part_name = nc.partition_id_tensor.name if nc.partition_id_tensor else None
    in_names, out_names, out_avals, zero_outs = [], [], [], []
    for alloc in nc.m.functions[0].allocations:
        if not isinstance(alloc, mb.MemoryLocationSet):
            continue
        name = alloc.memorylocations[0].name
        if alloc.kind == "ExternalInput":
            if name != part_name:
                in_names.append(name)
        elif alloc.kind == "ExternalOutput":
            shape = tuple(alloc.tensor_shape)
            dtype = mb.dt.np(alloc.dtype)
            out_names.append(name)
            out_avals.append(jax.core.ShapedArray(shape, dtype))
            zero_outs.append(np.zeros(shape, dtype))
    n_params = len(in_names)
    all_names = in_names + out_names
    if part_name is not None:
        all_names = all_names + [part_name]
    donate = tuple(range(n_params, n_params + len(out_names)))

    def _body(*args):
        operands = list(args)
        if part_name is not None:
            operands.append(bass2jax.partition_id_tensor())
        outs = bass2jax._bass_exec_p.bind(
            *operands,
            out_avals=tuple(out_avals),
            in_names=tuple(all_names),
            out_names=tuple(out_names),
            lowering_input_output_aliases=(),
            sim_require_finite=True,
            sim_require_nnan=True,
            nc=nc,
        )
        return tuple(outs)

    devices = jax.devices()[:N_CORES]
    mesh = Mesh(np.asarray(devices), ("core",))
    n_args = n_params + len(out_names)
    sharded = jax.jit(
        shard_map(
            _body,
            mesh=mesh,
            in_specs=(PartitionSpec("core"),) * n_args,
            out_specs=(PartitionSpec("core"),) * len(out_names),
            check_rep=False,
        ),
        donate_argnums=donate,
        keep_unused=True,
    )

    def run(in_maps):
        concat_in = [
            np.concatenate([np.asarray(m[name]) for m in in_maps], axis=0)
            for name in in_names
        ]
        concat_zeros = [
            np.zeros((N_CORES * z.shape[0], *z.shape[1:]), z.dtype)
            for z in zero_outs
        ]
        out_arrs = sharded(*concat_in, *concat_zeros)
        out_arrs = [np.asarray(a) for a in out_arrs]
        return [
            {
                name: out_arrs[i].reshape(N_CORES, *out_avals[i].shape)[c]
                for i, name in enumerate(out_names)
            }
            for c in range(N_CORES)
        ]

    return run


def host_layout(a: np.ndarray) -> np.ndarray:
    """[8, 16, 2048] fp32 -> per-core [128, 272] block-transposed with a
    +30 pad column after each row's 16 blocks."""
    a = a.reshape(N_CORES, ROWS, NB, 128).transpose(0, 3, 1, 2)
    out = np.full((N_CORES, 128, ROWS, NBP), PAD, dtype=np.float32)
    out[:, :, :, :NB] = a
    return np.ascontiguousarray(out.reshape(N_CORES, 128, NCOL))


def make_in_maps(input: np.ndarray, target: np.ndarray):
    x = host_layout(np.ascontiguousarray(input, dtype=np.float32).reshape(
        N_CORES, ROWS, L))
    t = host_layout(np.ascontiguousarray(target, dtype=np.float32).reshape(
        N_CORES, ROWS, L))
    if "w" not in _CACHE:
        _CACHE["w"] = np.ascontiguousarray(
            make_weights().reshape(128, 2 * NK * 128))
    w = _CACHE["w"]
    return [{"x": x[c], "t": t[c], "w": w} for c in range(N_CORES)]


def kernel(input: np.ndarray, target: np.ndarray) -> np.ndarray:
    if "run" not in _CACHE:
        _CACHE["nc"] = build()
        _CACHE["run"] = make_runner(_CACHE["nc"])

    results = _CACHE["run"](make_in_maps(input, target))
    acc = np.stack([r["acc"] for r in results])      # [cores, 128, 12]
    return finish(acc)


def finish(acc: np.ndarray) -> np.ndarray:
    per_k = acc.sum(axis=(0, 1), dtype=np.float64).reshape(2, NK).sum(0)
    counts = np.array([B * C * (L - k + 1) for k in KERNELS], dtype=np.float64)
    return np.float32((per_k / counts).sum())
